# revision 27
# baseline (speedup 1.0000x reference)
"""Trainium2 Bass kernel for the L1-distance attention + MLP-scaling model.

Math (per batch b):
  Wk = MLP(K), Wq = MLP(Q), Wo = MLPo(Q)
  Ks = K*Wk, Qs = Q*Wq
  score[k,q] = sum_d |Ks[k,d] - Qs[q,d]|
             = (Sq[q] - Sk[k]) + 2*sum_d relu(Ks[k,d] - Qs[q,d])
  attn = softmax_k(-(score^2)/2)      (softmax over keys)
  out = (attn^T @ V) * Wo

Sharding: 8 cores = 4 batches x 2 query-halves. Each core handles all 4096
keys and 2048 queries of its batch.

Device algorithm (per core), PE-bound by design (~96% tensor-engine busy):
  - MLPs computed in transposed layout (features on partitions).
  - Ks held as fp16 [128, nk] (two copies of the 64 features stacked), and
    per query pair (2 queries x 64 features on 128 partitions) one fp16
    relu(Ks - Qs) tile is produced per 2048-key half by DVE tensor_scalar
    in 4x mode, then reduced over features by PE against a shifted
    2-hot E matrix into 8 PSUM banks (512 keys each), accumulating over
    the 64 query pairs of a subtile.  The (Sq - Sk) rank-1 term is a final
    f32r matmul into the same banks.
  - Softmax: squares move scores out of PSUM per half (freeing banks for
    the next subtile), per-bank mins track the offset (their emission is
    deferred past the next rt stream to keep the in-order DVE queue
    clear), exp(min - s2) happens pre-transpose (the offset cancels in
    normalization), the [q,k] -> [k,q] transpose rides the otherwise-idle
    DMA xbar transpose engine (PE for the final subtile, where PE is
    idle), and context accumulates against fp16 V with an appended ones
    column so the denominator comes out as row 64.
  - Emission is software-pipelined across subtiles:
      score_loop(s+1,0) | tail_a(s) | score_fin(s+1,0) |
      score(s+1,1) | tail_b(s)
    so PE alternates score and softmax work without waiting on DVE/Act.

I/O: all per-core operands are packed into ONE flat f32 DRAM blob (the
dispatch path charges ~0.5ms per input tensor, not per byte; fp16 operands
are packed as bit pairs), and the output is returned as bf16 (halves the
device->host bytes, ~4ms) then upcast on host.

Precision: fp16 Ks/relu-terms/attn/V + bf16 output measure 3.0e-3 rel err
vs the f32 reference (budget 2e-2); fp16 keeps 10 mantissa bits which the
score->logit sensitivity (score ~ 4-9) comfortably tolerates, while bf16
relu terms would land at 1.8e-2.
"""
import sys
sys.path.insert(0, '/opt/trn_rl_repo')
import numpy as np
from contextlib import ExitStack

import concourse.bass as bass
import concourse.bacc as bacc
import concourse.tile as tile
from concourse import mybir
from concourse.bass_utils import run_bass_kernel_spmd

dt = mybir.dt
F32 = dt.float32
F32R = dt.float32r
BF16 = dt.bfloat16
F16 = dt.float16
ALU = mybir.AluOpType
AF = mybir.ActivationFunctionType
AX = mybir.AxisListType

B, NK, NQ, DK, DV, H = 4, 4096, 4096, 64, 64, 256
NCORES = 8
QSH = NQ // 2            # queries per core
NSUB = QSH // 128        # 16 q-subtiles of 128
KCH = NK // 128          # 32 key chunks
KB = NK // 512           # 8 psum banks of 512 keys
SQ2 = float(np.float32(1.0 / np.sqrt(2.0)))

# ---- packed input blob layout: name -> (partitions, free) ----
_SEGS = [
    ("kt", 64, NK), ("qt", 64, QSH), ("v1", 128, KCH * 65 // 2),
    ("w1", 64, H), ("w2a", 128, H), ("w2b", 128, H),
    ("w3a", 128, DK), ("w3b", 128, DK),
    ("b1c", 128, 2), ("b2c", 128, 2), ("b3c", 128, 1),
    ("u1", 64, H), ("u2a", 128, H), ("u2b", 128, H),
    ("u3a", 128, DK), ("u3b", 128, DK),
    ("c1c", 128, 2), ("c2c", 128, 2), ("c3c", 128, 1),
    ("em", 128, 96), ("iden", 128, 128),
]
_OFFS = {}
_tot = 0
for _n, _p, _f in _SEGS:
    _OFFS[_n] = _tot
    _tot += _p * _f
TOT = _tot

_cache = {}


def _build(nsub, reps=1):
    nc = bacc.Bacc("TRN2", target_bir_lowering=False, debug=False,
                   num_devices=NCORES)

    blob = nc.dram_tensor("blob", [1, TOT], F32, kind="ExternalInput").ap()
    O = nc.dram_tensor("o", [QSH, DV], BF16, kind="ExternalOutput").ap()

    segshape = dict((n, (p, f)) for n, p, f in _SEGS)

    def seg(name):
        p, f = segshape[name]
        o = _OFFS[name]
        return blob[0:1, o:o + p * f].rearrange("r (p f) -> (r p) f", p=p)

    with tile.TileContext(nc) as tc:
        with ExitStack() as ctx:
            if reps > 1:
                ctx.enter_context(tc.For_i(0, reps, 1))
            sb = ctx.enter_context(tc.tile_pool(name="sb", bufs=1))
            hp = ctx.enter_context(tc.tile_pool(name="hp", bufs=2))
            bp = ctx.enter_context(tc.tile_pool(name="bp", bufs=1))
            rp = ctx.enter_context(tc.tile_pool(name="rp", bufs=2))
            pp = ctx.enter_context(tc.tile_pool(name="pp", bufs=1, space="PSUM"))

            def psum(tag):
                return pp.tile([128, 512], F32, tag=tag, name=tag)

            # ---------- load inputs (sliced out of the single blob) ----------
            def load(name, d=F32):
                p, f = segshape[name]
                t = sb.tile([p, f], d, tag=name, name=name)
                src = seg(name)
                if d is not F32:
                    src = src.bitcast(d)
                nc.gpsimd.dma_start(t[:], src)
                return t

            # load order: K-MLP operands first (kt chunk 0 + weights +
            # biases) so compute starts ASAP; kt's remaining chunks stream
            # during compute; bulk operands go out on the sync HWDGE queue
            # so the two DMA paths issue descriptors in parallel
            p, f = segshape["kt"]
            kt_t = sb.tile([p, f], F32, tag="kt", name="kt")
            kt_src = seg("kt")
            nc.gpsimd.dma_start(kt_t[:, 0:512], kt_src[:, 0:512])
            w1_t = load("w1")
            b1_t = load("b1c"); b2_t = load("b2c"); b3_t = load("b3c")
            w2a_t = load("w2a"); w2b_t = load("w2b")
            w3a_t = load("w3a"); w3b_t = load("w3b")
            for c in range(1, NK // 512):
                nc.gpsimd.dma_start(kt_t[:, c * 512:(c + 1) * 512],
                                    kt_src[:, c * 512:(c + 1) * 512])
            qt_t = load("qt")
            u1_t = load("u1")
            u2a_t = load("u2a"); u2b_t = load("u2b")
            u3a_t = load("u3a"); u3b_t = load("u3b")
            c1_t = load("c1c"); c2_t = load("c2c"); c3_t = load("c3c")

            def loads(name, d=F32):
                p2, f2 = segshape[name]
                t = sb.tile([p2, f2], d, tag=name, name=name)
                src2 = seg(name)
                if d is not F32:
                    src2 = src2.bitcast(d)
                nc.sync.dma_start(t[:], src2)
                return t

            v1_t = loads("v1")
            v1b = v1_t[:].bitcast(F16)           # [128, KCH*65] fp16
            iden_t = loads("iden")
            idenb = sb.tile([128, 128], F16, tag="idenb", name="idenb")
            nc.vector.tensor_copy(idenb[:], iden_t[:])
            em_f = loads("em")
            em_t = em_f[:].bitcast(F16)       # [128, 192] fp16

            # ---------- MLPs (transposed layout: features on partitions) ----------
            # out_t[0:64, :] gets scaled product written in-place for ks2
            ks2 = sb.tile([128, NK], F16, tag="ks2")
            qsct = sb.tile([64, QSH], F32, tag="qsct")
            wot = sb.tile([64, QSH], F32, tag="wot")

            def mlp(x_t, T, l1, l2a, l2b, l3a, l3b, bb1, bb2, bb3, out_ap,
                    scale_by=None):
                # x_t: [64, T] input^T; writes MLP output^T (64 rows) to out_ap
                # if scale_by is given, writes (mlp_out * scale_by) instead
                for c in range(T // 512):
                    xc = x_t[:, c * 512:(c + 1) * 512]
                    pa, pb = psum("bank0"), psum("bank1")
                    nc.tensor.matmul(pa[:], l1[:, 0:128], xc, start=True, stop=True)
                    nc.tensor.matmul(pb[:], l1[:, 128:256], xc, start=True, stop=True)
                    h1a = hp.tile([128, 512], F32, tag="h1a")
                    h1b = hp.tile([128, 512], F32, tag="h1b")
                    nc.vector.tensor_scalar(h1a[:], pa[:], bb1[:, 0:1], 0.0,
                                            ALU.add, ALU.max)
                    nc.vector.tensor_scalar(h1b[:], pb[:], bb1[:, 1:2], 0.0,
                                            ALU.add, ALU.max)
                    pc, pd = psum("bank2"), psum("bank3")
                    nc.tensor.matmul(pc[:], l2a[:, 0:128], h1a[:], start=True, stop=False)
                    nc.tensor.matmul(pc[:], l2b[:, 0:128], h1b[:], start=False, stop=True)
                    nc.tensor.matmul(pd[:], l2a[:, 128:256], h1a[:], start=True, stop=False)
                    nc.tensor.matmul(pd[:], l2b[:, 128:256], h1b[:], start=False, stop=True)
                    h2a = hp.tile([128, 512], F32, tag="h2a")
                    h2b = hp.tile([128, 512], F32, tag="h2b")
                    nc.vector.tensor_scalar(h2a[:], pc[:], bb2[:, 0:1], 0.0,
                                            ALU.add, ALU.max)
                    nc.vector.tensor_scalar(h2b[:], pd[:], bb2[:, 1:2], 0.0,
                                            ALU.add, ALU.max)
                    pe_ = psum("bank4")
                    nc.tensor.matmul(pe_[0:64, :], l3a[:, 0:64], h2a[:], start=True, stop=False)
                    nc.tensor.matmul(pe_[0:64, :], l3b[:, 0:64], h2b[:], start=False, stop=True)
                    oc = out_ap[:, c * 512:(c + 1) * 512]
                    if scale_by is None:
                        nc.vector.tensor_scalar(oc, pe_[0:64, :], bb3[0:64, 0:1],
                                                None, ALU.add)
                    else:
                        w_sb = hp.tile([64, 512], F32, tag="wsb")
                        nc.vector.tensor_scalar(w_sb[:], pe_[0:64, :], bb3[0:64, 0:1],
                                                None, ALU.add)
                        nc.vector.tensor_tensor(
                            oc, w_sb[:], scale_by[:, c * 512:(c + 1) * 512],
                            ALU.mult)

            mlp(kt_t, NK, w1_t, w2a_t, w2b_t, w3a_t, w3b_t, b1_t, b2_t, b3_t,
                ks2[0:64, :], scale_by=kt_t)          # Ks^T into ks2 top
            mlp(qt_t, QSH, w1_t, w2a_t, w2b_t, w3a_t, w3b_t, b1_t, b2_t, b3_t,
                qsct[:], scale_by=qt_t)               # Qs^T
            mlp(qt_t, QSH, u1_t, u2a_t, u2b_t, u3a_t, u3b_t, c1_t, c2_t, c3_t,
                wot[:])                               # Wo^T

            # duplicate Ks^T into bottom half of ks2
            nc.gpsimd.dma_start(ks2[64:128, :], ks2[0:64, :])

            # qs2cols: [128, QSH/2]; col j = [Qs[2j,:] ; Qs[2j+1,:]]
            qs2 = sb.tile([128, QSH // 2], F32, tag="qs2")
            qv = qsct[:].rearrange("p (s h j) -> p s h j", h=2, j=64)
            qd = qs2[:].rearrange("p (s j) -> p s j", j=64)
            nc.gpsimd.dma_start(qd[0:64, :, :], qv[:, :, 0, :])
            nc.gpsimd.dma_start(qd[64:128, :, :], qv[:, :, 1, :])

            # row sums Sq [1, QSH], Sk [1, NK] via ones-vector matmuls (fp32)
            ones64 = sb.tile([64, 1], F32, tag="ones64")
            nc.vector.memset(ones64[:], 1.0)
            ones16 = sb.tile([64, 1], F16, tag="ones16", name="ones16")
            nc.vector.memset(ones16[:], 1.0)
            l_r1 = sb.tile([2, QSH], F32R, tag="l_r1")
            sq_p = psum("bank5")
            for c in range(QSH // 512):
                nc.tensor.matmul(sq_p[0:1, :], ones64[:],
                                 qsct[:, c * 512:(c + 1) * 512],
                                 start=True, stop=True)
                # psum rows are in natural query order -> straight into l_r1
                nc.vector.tensor_copy(l_r1[0:1, c * 512:(c + 1) * 512],
                                      sq_p[0:1, :])
            rhs_r1 = sb.tile([2, KB * 512], F32R, tag="rhs_r1")
            nc.vector.memset(rhs_r1[0:1, :].bitcast(F32), 1.0)
            nc.gpsimd.dma_start(l_r1[1:2, :], rhs_r1[0:1, 0:QSH])
            sk_p = psum("bank6")
            for c in range(KB):
                nc.tensor.matmul(sk_p[0:1, :], ones16[:],
                                 ks2[0:64, c * 512:(c + 1) * 512],
                                 start=True, stop=True)
                skc = hp.tile([1, 512], F32R, tag="skc", name="skc")
                nc.vector.tensor_scalar(skc[0:1, :], sk_p[0:1, :], -1.0,
                                        None, ALU.mult)
                nc.gpsimd.dma_start(rhs_r1[1:2, c * 512:(c + 1) * 512], skc[0:1, :])

            # Wo natural layout [128, nsub*64] via PE transposes
            wo_nat = sb.tile([128, NSUB * 64], F32, tag="wo_nat")
            for s in range(nsub):
                pt = psum("bank7")
                nc.tensor.matmul(pt[:, 0:64], wot[:, s * 128:(s + 1) * 128],
                                 iden_t[0:64, 0:64], is_transpose=True,
                                 start=True, stop=True)
                nc.vector.tensor_copy(wo_nat[:, s * 64:(s + 1) * 64], pt[:, 0:64])

            out_stage = sb.tile([128, NSUB * 64], BF16, tag="out_stage")

            # ---------- main loop over query subtiles (software pipelined) ----
            # PSUM banks 0-3 hold half-0 scores, banks 4-7 half-1; each bank
            # is squared out to SBUF (s2h) right after its half finishes,
            # freeing it for the next subtile.  s2h/mcat are double-buffered
            # across subtiles (see the emission loop at the bottom for the
            # cross-subtile interleave).
            s2h_t = [bp.tile([128, NK], F32, tag=f"s2h{i}", name=f"s2h{i}")
                     for i in range(2)]
            mcat_t = [bp.tile([128, KB], F32, tag=f"mcat{i}", name=f"mcat{i}")
                      for i in range(2)]
            eqk = bp.tile([128, NK], F16, tag="eqk")
            attn = bp.tile([128, NK], F16, tag="attn")

            def score_loop(s, hf):
                ko = hf * (NK // 2)
                banks = [psum(f"bank{hf * 4 + kk}") for kk in range(4)]
                for jp in range(64):
                    col = s * 64 + jp
                    lw = em_t[:, 63 - jp:191 - jp]
                    # all-fp16 rt production runs on DVE alone in 4x mode
                    # (~594ns per 2048-wide op vs ~870ns/iter PE consumption)
                    rt = rp.tile([128, 2048], F16, tag="rhs", name="rhs")
                    nc.vector.tensor_scalar(rt[:], ks2[:, ko:ko + 2048],
                                            qs2[:, col:col + 1], 0.0,
                                            ALU.subtract, ALU.max)
                    st = (jp == 0)
                    for kk in range(4):
                        nc.tensor.matmul(banks[kk][:], lw,
                                         rt[:, kk * 512:(kk + 1) * 512],
                                         start=st, stop=False)
                return banks

            def score_fin(s, hf, banks):
                # rank-1 (Sq - Sk) term closes each bank; square/2 out of
                # PSUM (frees the bank for the next subtile)
                s2h = s2h_t[s % 2]
                for kk in range(4):
                    kb = hf * 4 + kk
                    nc.tensor.matmul(banks[kk][:],
                                     l_r1[:, s * 128:(s + 1) * 128],
                                     rhs_r1[:, kb * 512:(kb + 1) * 512],
                                     start=False, stop=True)
                    nc.scalar.activation(s2h[:, kb * 512:(kb + 1) * 512],
                                         banks[kk][:], AF.Square, scale=SQ2)

            def score_mins(s, hf):
                # per-bank min of score^2/2 -> mcat (emitted AFTER the next
                # rt stream so these never sit ahead of rt work in the
                # in-order DVE queue)
                s2h = s2h_t[s % 2]
                mcat = mcat_t[s % 2]
                for kk in range(4):
                    kb = hf * 4 + kk
                    nc.vector.tensor_reduce(mcat[:, kb:kb + 1],
                                            s2h[:, kb * 512:(kb + 1) * 512],
                                            AX.X, ALU.min)

            def score_half(s, hf):
                score_fin(s, hf, score_loop(s, hf))
                score_mins(s, hf)

            def tail_a(s):
                # softmax front half: offset, exp, transpose into [k,q]
                s2h = s2h_t[s % 2]
                mcat = mcat_t[s % 2]
                m2h = hp.tile([128, 1], F32, tag="m2h")
                nc.vector.tensor_reduce(m2h[:], mcat[:], AX.X, ALU.min)
                if s < nsub - 1:
                    # exp(m2h - s2h) in one op (the m2h offset cancels in
                    # the softmax normalization; it is for fp range safety)
                    nc.scalar.activation(eqk[:], s2h[:], AF.Exp,
                                         bias=m2h[:, 0:1], scale=-1.0)
                    # [q,k] -> [k,q] per 128-key chunk via the DMA xbar
                    # transpose engine (frees PE and PSUM entirely)
                    for c in range(KCH):
                        nc.sync.dma_start_transpose(
                            attn[:, c * 128:(c + 1) * 128],
                            eqk[:, c * 128:(c + 1) * 128])
                else:
                    # final subtile: nothing left to overlap, so shorten the
                    # serial chain instead — chunked exp feeding PE
                    # transposes (PE idle here) group by group
                    for g in range(KCH // 4):
                        nc.scalar.activation(
                            eqk[:, g * 512:(g + 1) * 512],
                            s2h[:, g * 512:(g + 1) * 512], AF.Exp,
                            bias=m2h[:, 0:1], scale=-1.0)
                        put = pp.tile([128, 512], F16,
                                      tag=f"bank{5 + g % 3}", name="put")
                        for ci in range(4):
                            c = g * 4 + ci
                            nc.tensor.matmul(put[:, ci * 128:(ci + 1) * 128],
                                             eqk[:, c * 128:(c + 1) * 128],
                                             idenb[:], is_transpose=True,
                                             start=True, stop=True)
                        nc.vector.tensor_copy(
                            attn[:, g * 512:(g + 1) * 512], put[:])

            def tail_b(s):
                # softmax back half: context matmul + normalize + Wo scale
                pctx = pp.tile([65, 128], F32, tag="bank4", name="pctx")
                for c in range(KCH):
                    nc.tensor.matmul(pctx[:], v1b[:, c * 65:(c + 1) * 65],
                                     attn[:, c * 128:(c + 1) * 128],
                                     start=(c == 0), stop=(c == KCH - 1))
                ctxs = hp.tile([65, 128], F32, tag="ctxs")
                nc.scalar.activation(ctxs[:], pctx[:], AF.Copy)
                pctx2 = pp.tile([128, 65], F32, tag="bank4", name="pctx2")
                nc.tensor.matmul(pctx2[:], ctxs[:], iden_t[0:65, 0:65],
                                 is_transpose=True, start=True, stop=True)
                rcp = hp.tile([128, 1], F32, tag="rcp")
                nc.vector.reciprocal(rcp[:], pctx2[:, 64:65])
                tmpo = hp.tile([128, 64], F32, tag="tmpo")
                nc.vector.tensor_scalar(tmpo[:], pctx2[:, 0:64], rcp[:, 0:1],
                                        None, ALU.mult)
                nc.vector.tensor_tensor(out_stage[:, s * 64:(s + 1) * 64],
                                        tmpo[:], wo_nat[:, s * 64:(s + 1) * 64],
                                        ALU.mult)

            # emission order interleaves the next subtile's two score halves
            # with this subtile's softmax halves so PE never waits on the
            # DVE/Act softmax chain; tail_a sits between score_loop and
            # score_fin so its exp runs on Act ahead of the next squares,
            # and each half's mins are deferred past the following rt
            # stream to keep the DVE queue clear
            score_half(0, 0)
            score_half(0, 1)
            deferred_mins = None
            for s in range(nsub):
                if s + 1 < nsub:
                    b0 = score_loop(s + 1, 0)
                    if deferred_mins is not None:
                        score_mins(*deferred_mins)
                    tail_a(s)
                    score_fin(s + 1, 0, b0)
                    b1 = score_loop(s + 1, 1)
                    score_mins(s + 1, 0)
                    score_fin(s + 1, 1, b1)
                    deferred_mins = (s + 1, 1)
                    tail_b(s)
                else:
                    if deferred_mins is not None:
                        score_mins(*deferred_mins)
                        deferred_mins = None
                    tail_a(s)
                    tail_b(s)

            ov = O.rearrange("(s p) f -> p s f", p=128)
            sv = out_stage[:].rearrange("p (s f) -> p s f", f=64)
            nc.sync.dma_start(ov[:, 0:nsub, :], sv[:, 0:nsub, :])

    nc.compile()
    return nc


def _host_prep(inputs, core, nsub):
    """Build the per-core packed blob (host-side layout prep only)."""
    b = core // 2
    qh = core % 2
    K = inputs["KEY"][b]                      # [NK, 64]
    Q = inputs["QUERY"][b][qh * QSH:(qh + 1) * QSH]
    V = inputs["VALUE"][b]
    v1 = np.concatenate([V, np.ones((NK, 1), np.float32)], axis=1)  # [NK, 65]
    em = np.zeros((128, 192), np.float32)
    em[0:64, 63] = 2.0
    em[64:128, 127] = 2.0
    m = {
        "kt": np.ascontiguousarray(K.T),
        "qt": np.ascontiguousarray(Q.T),
        "v1": np.ascontiguousarray(
            v1.reshape(KCH, 128, 65).transpose(1, 0, 2).reshape(
                128, KCH * 65)).astype('<f2').view(np.float32),
        "w1": np.ascontiguousarray(inputs["W1_w"].T),
        "w2a": np.ascontiguousarray(inputs["W2_w"].T[0:128]),
        "w2b": np.ascontiguousarray(inputs["W2_w"].T[128:256]),
        "w3a": np.ascontiguousarray(inputs["W3_w"].T[0:128]),
        "w3b": np.ascontiguousarray(inputs["W3_w"].T[128:256]),
        "b1c": np.ascontiguousarray(inputs["W1_b"].reshape(2, 128).T),
        "b2c": np.ascontiguousarray(inputs["W2_b"].reshape(2, 128).T),
        "b3c": np.ascontiguousarray(
            np.pad(inputs["W3_b"], (0, 64)).reshape(1, 128).T),
        "u1": np.ascontiguousarray(inputs["Wo1_w"].T),
        "u2a": np.ascontiguousarray(inputs["Wo2_w"].T[0:128]),
        "u2b": np.ascontiguousarray(inputs["Wo2_w"].T[128:256]),
        "u3a": np.ascontiguousarray(inputs["Wo3_w"].T[0:128]),
        "u3b": np.ascontiguousarray(inputs["Wo3_w"].T[128:256]),
        "c1c": np.ascontiguousarray(inputs["Wo1_b"].reshape(2, 128).T),
        "c2c": np.ascontiguousarray(inputs["Wo2_b"].reshape(2, 128).T),
        "c3c": np.ascontiguousarray(
            np.pad(inputs["Wo3_b"], (0, 64)).reshape(1, 128).T),
        "em": em.astype('<f2').view(np.float32),
        "iden": np.eye(128, dtype=np.float32),
    }
    blob = np.empty(TOT, np.float32)
    for n, p, f in _SEGS:
        blob[_OFFS[n]:_OFFS[n] + p * f] = np.asarray(
            m[n], dtype=np.float32).ravel()
    return {"blob": blob}


def run(inputs, nsub=NSUB, trace=False):
    if nsub not in _cache:
        _cache[nsub] = _build(nsub)
    nc = _cache[nsub]
    in_maps = [_host_prep(inputs, c, nsub) for c in range(NCORES)]
    res = run_bass_kernel_spmd(nc, in_maps, list(range(NCORES)), trace=trace)
    out = np.zeros((B, NQ, DV), np.float32)
    for c in range(NCORES):
        b, qh = c // 2, c % 2
        out[b, qh * QSH:qh * QSH + nsub * 128] = \
            res.results[c]["o"][0:nsub * 128].astype(np.float32)
    return out, res


def kernel(**inputs):
    out, _ = run(inputs)
    return out


# revision 29
# speedup vs baseline: 1.1258x; 1.1258x over previous
"""Trainium2 Bass kernel for the L1-distance attention + MLP-scaling model.

Math (per batch b):
  Wk = MLP(K), Wq = MLP(Q), Wo = MLPo(Q)
  Ks = K*Wk, Qs = Q*Wq
  score[k,q] = sum_d |Ks[k,d] - Qs[q,d]|
             = (Sq[q] - Sk[k]) + 2*sum_d relu(Ks[k,d] - Qs[q,d])
  attn = softmax_k(-(score^2)/2)      (softmax over keys)
  out = (attn^T @ V) * Wo

Sharding: 8 cores = 4 batches x 2 query-halves. Each core handles all 4096
keys and 2048 queries of its batch.

Device algorithm (per core), PE-bound by design (~96% tensor-engine busy):
  - MLPs computed in transposed layout (features on partitions).
  - Ks held as fp16 [128, nk] (two copies of the 64 features stacked), and
    per query pair (2 queries x 64 features on 128 partitions) one fp16
    relu(Ks - Qs) tile is produced per 2048-key half by DVE tensor_scalar
    in 4x mode, then reduced over features by PE against a shifted
    2-hot E matrix into 8 PSUM banks (512 keys each), accumulating over
    the 64 query pairs of a subtile.  The (Sq - Sk) rank-1 term is a final
    f32r matmul into the same banks.
  - Softmax: squares move scores out of PSUM per half (freeing banks for
    the next subtile), per-bank mins track the offset (their emission is
    deferred past the next rt stream to keep the in-order DVE queue
    clear), exp(min - s2) happens pre-transpose (the offset cancels in
    normalization), the [q,k] -> [k,q] transpose rides the otherwise-idle
    DMA xbar transpose engine (PE for the final subtile, where PE is
    idle), and context accumulates against fp16 V with an appended ones
    column so the denominator comes out as row 64.
  - Emission is software-pipelined across subtiles:
      score_loop(s+1,0) | tail_a(s) | score_fin(s+1,0) |
      score(s+1,1) | tail_b(s)
    so PE alternates score and softmax work without waiting on DVE/Act.

I/O: all per-core operands are packed into ONE flat f32 DRAM blob (the
dispatch path charges ~0.5ms per input tensor, not per byte; fp16 operands
are packed as bit pairs), and the output is returned as bf16 (halves the
device->host bytes, ~4ms) then upcast on host.

Precision: fp16 Ks/relu-terms/attn/V + bf16 output measure 3.0e-3 rel err
vs the f32 reference (budget 2e-2); fp16 keeps 10 mantissa bits which the
score->logit sensitivity (score ~ 4-9) comfortably tolerates, while bf16
relu terms would land at 1.8e-2.
"""
import sys
sys.path.insert(0, '/opt/trn_rl_repo')
import numpy as np
from contextlib import ExitStack

import concourse.bass as bass
import concourse.bacc as bacc
import concourse.tile as tile
from concourse import mybir
from concourse.bass_utils import run_bass_kernel_spmd

dt = mybir.dt
F32 = dt.float32
F32R = dt.float32r
BF16 = dt.bfloat16
F16 = dt.float16
ALU = mybir.AluOpType
AF = mybir.ActivationFunctionType
AX = mybir.AxisListType

B, NK, NQ, DK, DV, H = 4, 4096, 4096, 64, 64, 256
NCORES = 8
QSH = NQ // 2            # queries per core
NSUB = QSH // 128        # 16 q-subtiles of 128
KCH = NK // 128          # 32 key chunks
KB = NK // 512           # 8 psum banks of 512 keys
SQ2 = float(np.float32(1.0 / np.sqrt(2.0)))

# ---- packed input blob layout: name -> (partitions, free) ----
_SEGS = [
    ("kt", 64, NK), ("qt", 64, QSH), ("v1", 128, KCH * 65 // 2),
    ("w1", 64, H), ("w2a", 128, H), ("w2b", 128, H),
    ("w3a", 128, DK), ("w3b", 128, DK),
    ("b1c", 128, 2), ("b2c", 128, 2), ("b3c", 128, 1),
    ("u1", 64, H), ("u2a", 128, H), ("u2b", 128, H),
    ("u3a", 128, DK), ("u3b", 128, DK),
    ("c1c", 128, 2), ("c2c", 128, 2), ("c3c", 128, 1),
    ("em", 128, 96), ("iden", 128, 128),
]
_OFFS = {}
_tot = 0
for _n, _p, _f in _SEGS:
    _OFFS[_n] = _tot
    _tot += _p * _f
TOT = _tot

_cache = {}


def _build(nsub, reps=1):
    nc = bacc.Bacc("TRN2", target_bir_lowering=False, debug=False,
                   num_devices=NCORES)

    blob = nc.dram_tensor("blob", [1, TOT], F32, kind="ExternalInput").ap()
    O = nc.dram_tensor("o", [QSH, DV], BF16, kind="ExternalOutput").ap()

    segshape = dict((n, (p, f)) for n, p, f in _SEGS)

    def seg(name):
        p, f = segshape[name]
        o = _OFFS[name]
        return blob[0:1, o:o + p * f].rearrange("r (p f) -> (r p) f", p=p)

    with tile.TileContext(nc) as tc:
        with ExitStack() as ctx:
            if reps > 1:
                ctx.enter_context(tc.For_i(0, reps, 1))
            sb = ctx.enter_context(tc.tile_pool(name="sb", bufs=1))
            hp = ctx.enter_context(tc.tile_pool(name="hp", bufs=2))
            bp = ctx.enter_context(tc.tile_pool(name="bp", bufs=1))
            rp = ctx.enter_context(tc.tile_pool(name="rp", bufs=2))
            pp = ctx.enter_context(tc.tile_pool(name="pp", bufs=1, space="PSUM"))

            def psum(tag):
                return pp.tile([128, 512], F32, tag=tag, name=tag)

            # ---------- load inputs (sliced out of the single blob) ----------
            def load(name, d=F32):
                p, f = segshape[name]
                t = sb.tile([p, f], d, tag=name, name=name)
                src = seg(name)
                if d is not F32:
                    src = src.bitcast(d)
                nc.gpsimd.dma_start(t[:], src)
                return t

            # load order: K-MLP operands first (kt chunk 0 + weights +
            # biases) so compute starts ASAP; kt's remaining chunks stream
            # during compute; bulk operands go out on the sync HWDGE queue
            # so the two DMA paths issue descriptors in parallel
            p, f = segshape["kt"]
            kt_t = sb.tile([p, f], F32, tag="kt", name="kt")
            kt_src = seg("kt")
            nc.gpsimd.dma_start(kt_t[:, 0:512], kt_src[:, 0:512])
            w1_t = load("w1")
            b1_t = load("b1c"); b2_t = load("b2c"); b3_t = load("b3c")
            w2a_t = load("w2a"); w2b_t = load("w2b")
            w3a_t = load("w3a"); w3b_t = load("w3b")
            for c in range(1, NK // 512):
                nc.gpsimd.dma_start(kt_t[:, c * 512:(c + 1) * 512],
                                    kt_src[:, c * 512:(c + 1) * 512])
            qt_t = load("qt")
            u1_t = load("u1")
            u2a_t = load("u2a"); u2b_t = load("u2b")
            u3a_t = load("u3a"); u3b_t = load("u3b")
            c1_t = load("c1c"); c2_t = load("c2c"); c3_t = load("c3c")

            def loads(name, d=F32):
                p2, f2 = segshape[name]
                t = sb.tile([p2, f2], d, tag=name, name=name)
                src2 = seg(name)
                if d is not F32:
                    src2 = src2.bitcast(d)
                nc.sync.dma_start(t[:], src2)
                return t

            v1_t = loads("v1")
            v1b = v1_t[:].bitcast(F16)           # [128, KCH*65] fp16
            iden_t = loads("iden")
            idenb = sb.tile([128, 128], F16, tag="idenb", name="idenb")
            nc.vector.tensor_copy(idenb[:], iden_t[:])
            em_f = loads("em")
            em_t = em_f[:].bitcast(F16)       # [128, 192] fp16

            # ---------- MLPs (transposed layout: features on partitions) ----------
            # out_t[0:64, :] gets scaled product written in-place for ks2
            ks2 = sb.tile([128, NK], F16, tag="ks2")
            qsct = sb.tile([64, QSH], F32, tag="qsct")
            wot = sb.tile([64, QSH], F32, tag="wot")

            def mlp(x_t, T, l1, l2a, l2b, l3a, l3b, bb1, bb2, bb3, out_ap,
                    scale_by=None):
                # x_t: [64, T] input^T; writes MLP output^T (64 rows) to out_ap
                # if scale_by is given, writes (mlp_out * scale_by) instead
                for c in range(T // 512):
                    xc = x_t[:, c * 512:(c + 1) * 512]
                    pa, pb = psum("bank0"), psum("bank1")
                    nc.tensor.matmul(pa[:], l1[:, 0:128], xc, start=True, stop=True)
                    nc.tensor.matmul(pb[:], l1[:, 128:256], xc, start=True, stop=True)
                    h1a = hp.tile([128, 512], F32, tag="h1a")
                    h1b = hp.tile([128, 512], F32, tag="h1b")
                    nc.vector.tensor_scalar(h1a[:], pa[:], bb1[:, 0:1], 0.0,
                                            ALU.add, ALU.max)
                    nc.vector.tensor_scalar(h1b[:], pb[:], bb1[:, 1:2], 0.0,
                                            ALU.add, ALU.max)
                    pc, pd = psum("bank2"), psum("bank3")
                    nc.tensor.matmul(pc[:], l2a[:, 0:128], h1a[:], start=True, stop=False)
                    nc.tensor.matmul(pc[:], l2b[:, 0:128], h1b[:], start=False, stop=True)
                    nc.tensor.matmul(pd[:], l2a[:, 128:256], h1a[:], start=True, stop=False)
                    nc.tensor.matmul(pd[:], l2b[:, 128:256], h1b[:], start=False, stop=True)
                    h2a = hp.tile([128, 512], F32, tag="h2a")
                    h2b = hp.tile([128, 512], F32, tag="h2b")
                    nc.vector.tensor_scalar(h2a[:], pc[:], bb2[:, 0:1], 0.0,
                                            ALU.add, ALU.max)
                    nc.vector.tensor_scalar(h2b[:], pd[:], bb2[:, 1:2], 0.0,
                                            ALU.add, ALU.max)
                    pe_ = psum("bank4")
                    nc.tensor.matmul(pe_[0:64, :], l3a[:, 0:64], h2a[:], start=True, stop=False)
                    nc.tensor.matmul(pe_[0:64, :], l3b[:, 0:64], h2b[:], start=False, stop=True)
                    oc = out_ap[:, c * 512:(c + 1) * 512]
                    if scale_by is None:
                        nc.vector.tensor_scalar(oc, pe_[0:64, :], bb3[0:64, 0:1],
                                                None, ALU.add)
                    else:
                        w_sb = hp.tile([64, 512], F32, tag="wsb")
                        nc.vector.tensor_scalar(w_sb[:], pe_[0:64, :], bb3[0:64, 0:1],
                                                None, ALU.add)
                        nc.vector.tensor_tensor(
                            oc, w_sb[:], scale_by[:, c * 512:(c + 1) * 512],
                            ALU.mult)

            mlp(kt_t, NK, w1_t, w2a_t, w2b_t, w3a_t, w3b_t, b1_t, b2_t, b3_t,
                ks2[0:64, :], scale_by=kt_t)          # Ks^T into ks2 top
            mlp(qt_t, QSH, w1_t, w2a_t, w2b_t, w3a_t, w3b_t, b1_t, b2_t, b3_t,
                qsct[:], scale_by=qt_t)               # Qs^T
            mlp(qt_t, QSH, u1_t, u2a_t, u2b_t, u3a_t, u3b_t, c1_t, c2_t, c3_t,
                wot[:])                               # Wo^T

            # duplicate Ks^T into bottom half of ks2
            nc.gpsimd.dma_start(ks2[64:128, :], ks2[0:64, :])

            # qs2cols: [128, QSH/2]; col j = [Qs[2j,:] ; Qs[2j+1,:]]
            qs2 = sb.tile([128, QSH // 2], F32, tag="qs2")
            qv = qsct[:].rearrange("p (s h j) -> p s h j", h=2, j=64)
            qd = qs2[:].rearrange("p (s j) -> p s j", j=64)
            nc.gpsimd.dma_start(qd[0:64, :, :], qv[:, :, 0, :])
            nc.gpsimd.dma_start(qd[64:128, :, :], qv[:, :, 1, :])

            # row sums Sq [1, QSH], Sk [1, NK] via ones-vector matmuls (fp32)
            ones64 = sb.tile([64, 1], F32, tag="ones64")
            nc.vector.memset(ones64[:], 1.0)
            ones16 = sb.tile([64, 1], F16, tag="ones16", name="ones16")
            nc.vector.memset(ones16[:], 1.0)
            l_r1 = sb.tile([2, QSH], F32R, tag="l_r1")
            sq_p = psum("bank5")
            for c in range(QSH // 512):
                nc.tensor.matmul(sq_p[0:1, :], ones64[:],
                                 qsct[:, c * 512:(c + 1) * 512],
                                 start=True, stop=True)
                # psum rows are in natural query order -> straight into l_r1
                nc.vector.tensor_copy(l_r1[0:1, c * 512:(c + 1) * 512],
                                      sq_p[0:1, :])
            rhs_r1 = sb.tile([2, KB * 512], F32R, tag="rhs_r1")
            nc.vector.memset(rhs_r1[0:1, :].bitcast(F32), 1.0)
            nc.gpsimd.dma_start(l_r1[1:2, :], rhs_r1[0:1, 0:QSH])
            sk_p = psum("bank6")
            for c in range(KB):
                nc.tensor.matmul(sk_p[0:1, :], ones16[:],
                                 ks2[0:64, c * 512:(c + 1) * 512],
                                 start=True, stop=True)
                skc = hp.tile([1, 512], F32R, tag="skc", name="skc")
                nc.vector.tensor_scalar(skc[0:1, :], sk_p[0:1, :], -1.0,
                                        None, ALU.mult)
                nc.gpsimd.dma_start(rhs_r1[1:2, c * 512:(c + 1) * 512], skc[0:1, :])

            # Wo natural layout [128, nsub*64] via PE transposes
            wo_nat = sb.tile([128, NSUB * 64], F32, tag="wo_nat")
            for s in range(nsub):
                pt = psum("bank7")
                nc.tensor.matmul(pt[:, 0:64], wot[:, s * 128:(s + 1) * 128],
                                 iden_t[0:64, 0:64], is_transpose=True,
                                 start=True, stop=True)
                nc.vector.tensor_copy(wo_nat[:, s * 64:(s + 1) * 64], pt[:, 0:64])

            out_stage = sb.tile([128, NSUB * 64], BF16, tag="out_stage")

            # ---------- main loop over query subtiles (software pipelined) ----
            # PSUM banks 0-3 hold half-0 scores, banks 4-7 half-1; each bank
            # is squared out to SBUF (s2h) right after its half finishes,
            # freeing it for the next subtile.  s2h/mcat are double-buffered
            # across subtiles (see the emission loop at the bottom for the
            # cross-subtile interleave).
            s2h_t = [bp.tile([128, NK], F32, tag=f"s2h{i}", name=f"s2h{i}")
                     for i in range(2)]
            mcat_t = [bp.tile([128, KB], F32, tag=f"mcat{i}", name=f"mcat{i}")
                      for i in range(2)]
            eqk = bp.tile([128, NK], F16, tag="eqk")
            attn = bp.tile([128, NK], F16, tag="attn")

            def score_loop(s, hf):
                ko = hf * (NK // 2)
                banks = [psum(f"bank{hf * 4 + kk}") for kk in range(4)]
                for jp in range(64):
                    col = s * 64 + jp
                    lw = em_t[:, 63 - jp:191 - jp]
                    # all-fp16 rt production runs on DVE alone in 4x mode
                    # (~594ns per 2048-wide op vs ~870ns/iter PE consumption)
                    rt = rp.tile([128, 2048], F16, tag="rhs", name="rhs")
                    nc.vector.tensor_scalar(rt[:], ks2[:, ko:ko + 2048],
                                            qs2[:, col:col + 1], 0.0,
                                            ALU.subtract, ALU.max)
                    st = (jp == 0)
                    for kk in range(4):
                        nc.tensor.matmul(banks[kk][:], lw,
                                         rt[:, kk * 512:(kk + 1) * 512],
                                         start=st, stop=False)
                return banks

            def score_fin(s, hf, banks):
                # rank-1 (Sq - Sk) term closes each bank; square/2 out of
                # PSUM (frees the bank for the next subtile)
                s2h = s2h_t[s % 2]
                for kk in range(4):
                    kb = hf * 4 + kk
                    nc.tensor.matmul(banks[kk][:],
                                     l_r1[:, s * 128:(s + 1) * 128],
                                     rhs_r1[:, kb * 512:(kb + 1) * 512],
                                     start=False, stop=True)
                    nc.scalar.activation(s2h[:, kb * 512:(kb + 1) * 512],
                                         banks[kk][:], AF.Square, scale=SQ2)

            def score_mins(s, hf):
                # per-bank min of score^2/2 -> mcat (emitted AFTER the next
                # rt stream so these never sit ahead of rt work in the
                # in-order DVE queue)
                s2h = s2h_t[s % 2]
                mcat = mcat_t[s % 2]
                for kk in range(4):
                    kb = hf * 4 + kk
                    nc.vector.tensor_reduce(mcat[:, kb:kb + 1],
                                            s2h[:, kb * 512:(kb + 1) * 512],
                                            AX.X, ALU.min)

            def score_half(s, hf):
                score_fin(s, hf, score_loop(s, hf))
                score_mins(s, hf)

            def tail_a(s):
                # softmax front half: offset, exp, transpose into [k,q]
                s2h = s2h_t[s % 2]
                mcat = mcat_t[s % 2]
                m2h = hp.tile([128, 1], F32, tag="m2h")
                nc.vector.tensor_reduce(m2h[:], mcat[:], AX.X, ALU.min)
                if s < nsub - 1:
                    # exp(m2h - s2h) in one op (the m2h offset cancels in
                    # the softmax normalization; it is for fp range safety)
                    nc.scalar.activation(eqk[:], s2h[:], AF.Exp,
                                         bias=m2h[:, 0:1], scale=-1.0)
                    # [q,k] -> [k,q] per 128-key chunk via the DMA xbar
                    # transpose engine (frees PE and PSUM entirely)
                    for c in range(KCH):
                        nc.sync.dma_start_transpose(
                            attn[:, c * 128:(c + 1) * 128],
                            eqk[:, c * 128:(c + 1) * 128])
                else:
                    # final subtile: nothing left to overlap, so shorten the
                    # serial chain instead — chunked exp feeding PE
                    # transposes (PE idle here) group by group
                    for g in range(KCH // 4):
                        nc.scalar.activation(
                            eqk[:, g * 512:(g + 1) * 512],
                            s2h[:, g * 512:(g + 1) * 512], AF.Exp,
                            bias=m2h[:, 0:1], scale=-1.0)
                        put = pp.tile([128, 512], F16,
                                      tag=f"bank{5 + g % 3}", name="put")
                        for ci in range(4):
                            c = g * 4 + ci
                            nc.tensor.matmul(put[:, ci * 128:(ci + 1) * 128],
                                             eqk[:, c * 128:(c + 1) * 128],
                                             idenb[:], is_transpose=True,
                                             start=True, stop=True)
                        nc.vector.tensor_copy(
                            attn[:, g * 512:(g + 1) * 512], put[:])

            def tail_b(s):
                # softmax back half: context matmul + normalize + Wo scale
                pctx = pp.tile([65, 128], F32, tag="bank4", name="pctx")
                for c in range(KCH):
                    nc.tensor.matmul(pctx[:], v1b[:, c * 65:(c + 1) * 65],
                                     attn[:, c * 128:(c + 1) * 128],
                                     start=(c == 0), stop=(c == KCH - 1))
                ctxs = hp.tile([65, 128], F32, tag="ctxs")
                nc.scalar.activation(ctxs[:], pctx[:], AF.Copy)
                pctx2 = pp.tile([128, 65], F32, tag="bank4", name="pctx2")
                nc.tensor.matmul(pctx2[:], ctxs[:], iden_t[0:65, 0:65],
                                 is_transpose=True, start=True, stop=True)
                rcp = hp.tile([128, 1], F32, tag="rcp")
                nc.vector.reciprocal(rcp[:], pctx2[:, 64:65])
                tmpo = hp.tile([128, 64], F32, tag="tmpo")
                nc.vector.tensor_scalar(tmpo[:], pctx2[:, 0:64], rcp[:, 0:1],
                                        None, ALU.mult)
                nc.vector.tensor_tensor(out_stage[:, s * 64:(s + 1) * 64],
                                        tmpo[:], wo_nat[:, s * 64:(s + 1) * 64],
                                        ALU.mult)

            # emission order interleaves the next subtile's two score halves
            # with this subtile's softmax halves so PE never waits on the
            # DVE/Act softmax chain; tail_a sits between score_loop and
            # score_fin so its exp runs on Act ahead of the next squares,
            # and each half's mins are deferred past the following rt
            # stream to keep the DVE queue clear
            score_half(0, 0)
            score_half(0, 1)
            deferred_mins = None
            for s in range(nsub):
                if s + 1 < nsub:
                    b0 = score_loop(s + 1, 0)
                    if deferred_mins is not None:
                        score_mins(*deferred_mins)
                    tail_a(s)
                    score_fin(s + 1, 0, b0)
                    b1 = score_loop(s + 1, 1)
                    score_mins(s + 1, 0)
                    score_fin(s + 1, 1, b1)
                    deferred_mins = (s + 1, 1)
                    tail_b(s)
                else:
                    if deferred_mins is not None:
                        score_mins(*deferred_mins)
                        deferred_mins = None
                    tail_a(s)
                    tail_b(s)

            ov = O.rearrange("(s p) f -> p s f", p=128)
            sv = out_stage[:].rearrange("p (s f) -> p s f", f=64)
            nc.sync.dma_start(ov[:, 0:nsub, :], sv[:, 0:nsub, :])

    nc.compile()
    return nc


def _host_prep(inputs, core, nsub):
    """Build the per-core packed blob (host-side layout prep only)."""
    b = core // 2
    qh = core % 2
    K = inputs["KEY"][b]                      # [NK, 64]
    Q = inputs["QUERY"][b][qh * QSH:(qh + 1) * QSH]
    V = inputs["VALUE"][b]
    v1 = np.concatenate([V, np.ones((NK, 1), np.float32)], axis=1)  # [NK, 65]
    em = np.zeros((128, 192), np.float32)
    em[0:64, 63] = 2.0
    em[64:128, 127] = 2.0
    m = {
        "kt": np.ascontiguousarray(K.T),
        "qt": np.ascontiguousarray(Q.T),
        "v1": np.ascontiguousarray(
            v1.reshape(KCH, 128, 65).transpose(1, 0, 2).reshape(
                128, KCH * 65)).astype('<f2').view(np.float32),
        "w1": np.ascontiguousarray(inputs["W1_w"].T),
        "w2a": np.ascontiguousarray(inputs["W2_w"].T[0:128]),
        "w2b": np.ascontiguousarray(inputs["W2_w"].T[128:256]),
        "w3a": np.ascontiguousarray(inputs["W3_w"].T[0:128]),
        "w3b": np.ascontiguousarray(inputs["W3_w"].T[128:256]),
        "b1c": np.ascontiguousarray(inputs["W1_b"].reshape(2, 128).T),
        "b2c": np.ascontiguousarray(inputs["W2_b"].reshape(2, 128).T),
        "b3c": np.ascontiguousarray(
            np.pad(inputs["W3_b"], (0, 64)).reshape(1, 128).T),
        "u1": np.ascontiguousarray(inputs["Wo1_w"].T),
        "u2a": np.ascontiguousarray(inputs["Wo2_w"].T[0:128]),
        "u2b": np.ascontiguousarray(inputs["Wo2_w"].T[128:256]),
        "u3a": np.ascontiguousarray(inputs["Wo3_w"].T[0:128]),
        "u3b": np.ascontiguousarray(inputs["Wo3_w"].T[128:256]),
        "c1c": np.ascontiguousarray(inputs["Wo1_b"].reshape(2, 128).T),
        "c2c": np.ascontiguousarray(inputs["Wo2_b"].reshape(2, 128).T),
        "c3c": np.ascontiguousarray(
            np.pad(inputs["Wo3_b"], (0, 64)).reshape(1, 128).T),
        "em": em.astype('<f2').view(np.float32),
        "iden": np.eye(128, dtype=np.float32),
    }
    blob = np.empty(TOT, np.float32)
    for n, p, f in _SEGS:
        blob[_OFFS[n]:_OFFS[n] + p * f] = np.asarray(
            m[n], dtype=np.float32).ravel()
    return {"blob": blob}


def run(inputs, nsub=NSUB, trace=False):
    if nsub not in _cache:
        _cache[nsub] = _build(nsub)
    nc = _cache[nsub]
    in_maps = [_host_prep(inputs, c, nsub) for c in range(NCORES)]
    res = run_bass_kernel_spmd(nc, in_maps, list(range(NCORES)), trace=trace)
    out = np.zeros((B, NQ, DV), np.float32)
    for c in range(NCORES):
        b, qh = c // 2, c % 2
        out[b, qh * QSH:qh * QSH + nsub * 128] = \
            res.results[c]["o"][0:nsub * 128].astype(np.float32)
    return out, res


def kernel(**inputs):
    out, _ = run(inputs)
    return out
